# revision 1
# baseline (speedup 1.0000x reference)
"""BoundaryAttentionModule Trainium2 kernel.

Shapes (hardcoded): b=4, c=256, h=w=64 (HW=4096), boundary 128x128,
mid=64, out_ch=256. 8 cores: core = (batch bi = core//2, key-half kh = core%2).

Math (exact reassociation of the reference):
  bm   = nearest-downsampled boundary map        [b, 4096]
  R    = relu(kw1f outer bm_khalf + beta)        [64, 2048]   (kw1f = key_w1*bn_inv)
  G    = (key_w2^T @ query_w) @ u                [64, 4096]
  E^T  = R^T @ G                                 [2048_k, 4096_j]  (logits tiny, no max sub)
  U    = exp(E^T), s[k] = sum_j U[k, j]
  Vt   = (u^T @ value_w^T)[k_half] / s * 8192    [2048, 256]
  P    = Vt^T @ U                                [256, 4096]  per-core partial (x8192)
host: out[bi] = (gamma/8192) * (P[2bi] + P[2bi+1]) + u[bi]

Energy matmuls in bf16 with K=64 contraction packed as concurrent
partition-half duos (R and G are host/device-duplicated into both
partition halves, so two j-slices of one k-tile run in the PE array
simultaneously).  The output matmul runs in fp8e4 DoubleRow (2 keys per
cell); the key axis is host-permuted pairwise so PSUM partitions
interleave — the permutation only reorders the contracted axis.
"""

import numpy as np

B, C, HW = 4, 256, 4096
KH = HW // 2          # 2048 keys per core
NK = KH // 128        # 16 k tiles
NP = NK // 2          # 8 k-tile pairs
MID = 64
VSCALE = 8192.0       # fp8 scaling of Vt (power of two; host divides gamma)

TRACE = False
TRACE_CORES = None
LAST_RESULTS = None

_BUILT = None


def _build():
    import concourse.bass as bass
    import concourse.tile as tile
    from concourse import bacc, mybir

    f32 = mybir.dt.float32
    bf16 = mybir.dt.bfloat16
    fp8 = mybir.dt.float8e4
    AF = mybir.ActivationFunctionType
    AX = mybir.AxisListType
    ALU = mybir.AluOpType

    nc = bacc.Bacc(
        "TRN2",
        target_bir_lowering=False,
        debug=False,
        enable_asserts=False,
        num_devices=8,
    )

    u_in = nc.dram_tensor("u_in", [C, HW], bf16, kind="ExternalInput").ap()
    uk_in = nc.dram_tensor("uk_in", [C, KH], bf16, kind="ExternalInput").ap()
    bmk_in = nc.dram_tensor("bmk_in", [1, KH], bf16, kind="ExternalInput").ap()
    # M2^T = [M; M]^T  [256, 128]  (duplicated so G lands in both halves)
    mt_in = nc.dram_tensor("mt_in", [C, 2 * MID], bf16, kind="ExternalInput").ap()
    vwt_in = nc.dram_tensor("vwt_in", [C, C], bf16, kind="ExternalInput").ap()
    kw1f_in = nc.dram_tensor("kw1f_in", [1, 2 * MID], bf16, kind="ExternalInput").ap()
    beta_in = nc.dram_tensor("beta_in", [2 * MID, 1], f32, kind="ExternalInput").ap()
    out_d = nc.dram_tensor("outp", [C, HW], f32, kind="ExternalOutput").ap()

    # j-chunking of the 4096 axis: two 1536 chunks + one 1024 chunk.
    # PSUM: "big" slots [128,1536] (3 banks) x2 bufs + "small" (1 bank) x2 = 8.
    CHUNKS = [(0, 1536), (1536, 1536), (3072, 1024)]
    C_CHUNKS = CHUNKS

    with tile.TileContext(nc) as tc:
        with (
            tc.tile_pool(name="sb", bufs=1) as sb,
            tc.tile_pool(name="ost", bufs=2) as osp,
            tc.tile_pool(name="ps", bufs=2, space="PSUM") as ps,
        ):
            # ---- weights / inputs; u on sync queue, the rest on gpsimd ----
            mt = sb.tile([128, 2 * MID], bf16, tag="mt", name="mt")
            nc.gpsimd.dma_start(mt[0:128, :], mt_in[0:128, :])
            mt1 = sb.tile([128, 2 * MID], bf16, tag="mt1", name="mt1")
            nc.gpsimd.dma_start(mt1[0:128, :], mt_in[128:256, :])
            kw1 = sb.tile([1, 2 * MID], bf16, tag="kw1", name="kw1")
            nc.gpsimd.dma_start(kw1[:], kw1f_in[:, :])
            betat = sb.tile([2 * MID, 1], f32, tag="betat", name="betat")
            nc.gpsimd.dma_start(betat[:], beta_in[:, :])
            bmk = sb.tile([1, KH], bf16, tag="bmk", name="bmk")
            nc.gpsimd.dma_start(bmk[:], bmk_in[:, :])
            u0 = sb.tile([128, HW], bf16, tag="u0", name="u0")
            u1 = sb.tile([128, HW], bf16, tag="u1", name="u1")
            for jo, w in CHUNKS:
                nc.sync.dma_start(u0[:, jo : jo + w], u_in[0:128, jo : jo + w])
                nc.sync.dma_start(u1[:, jo : jo + w], u_in[128:256, jo : jo + w])
            vwt0 = sb.tile([128, C], bf16, tag="vwt0", name="vwt0")
            nc.gpsimd.dma_start(vwt0[:], vwt_in[0:128, :])
            vwt1 = sb.tile([128, C], bf16, tag="vwt1", name="vwt1")
            nc.gpsimd.dma_start(vwt1[:], vwt_in[128:256, :])
            uk0 = sb.tile([128, KH], bf16, tag="uk0", name="uk0")
            nc.gpsimd.dma_start(uk0[:], uk_in[0:128, :])
            uk1 = sb.tile([128, KH], bf16, tag="uk1", name="uk1")
            nc.gpsimd.dma_start(uk1[:], uk_in[128:256, :])

            # ---- R2 = relu(kw1f2 outer bmk + beta2): both halves [128, 2048] ----
            R2 = sb.tile([128, KH], bf16, tag="R2", name="R2")
            for rc in range(KH // 512):
                pr = ps.tile([128, 512], f32, tag="small", bufs=2, name=f"pr{rc}")
                nc.tensor.matmul(
                    pr[:], kw1[:, :], bmk[:, rc * 512 : (rc + 1) * 512],
                    start=True, stop=True,
                )
                nc.scalar.activation(
                    R2[:, rc * 512 : (rc + 1) * 512], pr[:], AF.Relu,
                    bias=betat[:, 0:1],
                )

            # ---- G2 = M2 @ u (both halves) + first k-tile energy interleaved ----
            G2 = sb.tile([128, HW], bf16, tag="G2", name="G2")
            s_all = sb.tile([128, NK], f32, tag="s_all", name="s_all")
            rinv_all = sb.tile([128, NK], f32, tag="rinv", name="rinv_all")
            sp_tail = {}
            for kt in range(NK - 3, NK):
                sp_tail[kt] = sb.tile([128, 4], f32, tag=f"sp{kt}", name=f"sp{kt}")
            u_pairs = []
            for pair in range(NP):
                Up = sb.tile([128, 2 * HW], fp8, tag=f"Up{pair}", name=f"Up{pair}")
                u_pairs.append(Up)
            vtb = []
            for kt in range(NK):
                v = sb.tile([128, C], bf16, tag=f"vtb{kt}", name=f"vtb{kt}")
                vtb.append(v)
            vtsp = []
            for pair in range(NP):
                vp = sb.tile([128, 2 * C], fp8, tag=f"vtsp{pair}", name=f"vtsp{pair}")
                vtsp.append(vp)

            N_ACC = 3  # last k-tiles whose row-sum rides the ACT accumulator

            def energy_chunk(kt, ci):
                """Energy matmuls + exp for one (k-tile, j-chunk)."""
                pair, half = kt // 2, kt % 2
                Up = u_pairs[pair]
                accum_tail = kt >= NK - N_ACC
                jo, w = CHUNKS[ci]
                pe = ps.tile([128, 1536], f32, tag="big", name=f"pe{kt}_{jo}")
                nq = w // 512
                for q in range(0, nq, 2):
                    # concurrent partition-half duo (K=64 row groups)
                    js0 = jo + q * 512
                    nc.tensor.matmul(
                        pe[:, q * 512 : (q + 1) * 512],
                        R2[0:64, kt * 128 : (kt + 1) * 128],
                        G2[0:64, js0 : js0 + 512],
                        start=True, stop=True,
                    )
                    if q + 1 < nq:
                        js1 = jo + (q + 1) * 512
                        nc.tensor.matmul(
                            pe[:, (q + 1) * 512 : (q + 2) * 512],
                            R2[64:128, kt * 128 : (kt + 1) * 128],
                            G2[64:128, js1 : js1 + 512],
                            start=True, stop=True,
                        )
                nc.scalar.activation(
                    Up[:, half * HW + jo : half * HW + jo + w],
                    pe[:, 0:w], AF.Exp,
                    accum_out=(sp_tail[kt][:, ci : ci + 1] if accum_tail else None),
                )
                if accum_tail and ci == len(CHUNKS) - 1:
                    nc.vector.reduce_sum(
                        s_all[:, kt : kt + 1], sp_tail[kt][:, 0:3], axis=AX.X
                    )

            def ktile_epilogue(kt):
                """Row-sum (if not ACT-accumulated) + Vt matmul pair + scales."""
                pair, half = kt // 2, kt % 2
                if kt < NK - N_ACC:
                    nc.vector.reduce_sum(
                        s_all[:, kt : kt + 1],
                        u_pairs[pair][:, half * HW : (half + 1) * HW], axis=AX.X,
                    )
                pv = ps.tile([128, C], f32, tag="small", bufs=2, name=f"pv{kt}")
                ko = kt * 128
                nc.tensor.matmul(
                    pv[:], uk0[:, ko : ko + 128], vwt0[:, :], start=True, stop=False
                )
                nc.tensor.matmul(
                    pv[:], uk1[:, ko : ko + 128], vwt1[:, :], start=False, stop=True
                )
                nc.vector.tensor_copy(vtb[kt][:], pv[:])
                if half == 1:
                    nc.vector.reciprocal(
                        rinv_all[:, kt - 1 : kt + 1], s_all[:, kt - 1 : kt + 1]
                    )
                    for h2 in (0, 1):
                        nc.gpsimd.tensor_scalar(
                            vtsp[pair][:, h2 * C : (h2 + 1) * C],
                            vtb[kt - 1 + h2][:],
                            rinv_all[:, kt - 1 + h2 : kt + h2], VSCALE,
                            op0=ALU.mult, op1=ALU.mult,
                        )

            # G chunk production interleaved chunk-major with k-tiles 0 and 1,
            # so ACT has exp work while later G chunks are still being built
            for ci, (jo, w) in enumerate(CHUNKS):
                pg = ps.tile([128, 1536], f32, tag="big", name=f"pg{jo}")
                for q in range(w // 512):
                    sl = slice(q * 512, (q + 1) * 512)
                    js = jo + q * 512
                    nc.tensor.matmul(
                        pg[:, sl], mt[:, :], u0[:, js : js + 512],
                        start=True, stop=False,
                    )
                    nc.tensor.matmul(
                        pg[:, sl], mt1[:, :], u1[:, js : js + 512],
                        start=False, stop=True,
                    )
                nc.vector.tensor_copy(G2[:, jo : jo + w], pg[:, 0:w])
                energy_chunk(0, ci)
                energy_chunk(1, ci)
            ktile_epilogue(0)
            ktile_epilogue(1)
            for kt in range(2, NK):
                for ci in range(len(CHUNKS)):
                    energy_chunk(kt, ci)
                ktile_epilogue(kt)

            # ---- P = Vt^T @ U  (fp8 DoubleRow: 2 keys/cell) -> DRAM ----
            DR = mybir.MatmulPerfMode.DoubleRow
            for ct in range(2):
                for jg, (jo, w) in enumerate(C_CHUNKS):
                    po = ps.tile([128, 1536], f32, tag="big", name=f"po{ct}_{jg}")
                    for pair in range(NP):
                        lhsT = vtsp[pair].rearrange("p (i c) -> p i c", i=2)[
                            :, :, ct * 128 : (ct + 1) * 128
                        ]
                        for q in range(w // 512):
                            sl = slice(q * 512, (q + 1) * 512)
                            js = jo + q * 512
                            rhs = u_pairs[pair].rearrange("p (i j) -> p i j", i=2)[
                                :, :, js : js + 512
                            ]
                            nc.tensor.matmul(
                                po[:, sl], lhsT, rhs,
                                start=(pair == 0), stop=(pair == NP - 1),
                                perf_mode=DR,
                            )
                    ost = osp.tile([128, 1536], f32, tag="ost", name=f"ost{ct}_{jg}")
                    if ct == 1 and jg == len(C_CHUNKS) - 1:
                        # final group: split copy/DMA halves to shorten the tail
                        h = w // 2
                        nc.scalar.copy(ost[:, 0:h], po[:, 0:h])
                        nc.sync.dma_start(
                            out_d[ct * 128 : (ct + 1) * 128, jo : jo + h],
                            ost[:, 0:h],
                        )
                        nc.scalar.copy(ost[:, h:w], po[:, h:w])
                        nc.scalar.dma_start(
                            out_d[ct * 128 : (ct + 1) * 128, jo + h : jo + w],
                            ost[:, h:w],
                        )
                    else:
                        nc.scalar.copy(ost[:, 0:w], po[:, 0:w])
                        nc.sync.dma_start(
                            out_d[ct * 128 : (ct + 1) * 128, jo : jo + w],
                            ost[:, 0:w],
                        )

    nc.compile()
    return nc


def _get_built():
    global _BUILT
    if _BUILT is None:
        _BUILT = _build()
    return _BUILT


def _kperm():
    """Pairwise interleave within 256-key blocks: new index kt*128+q maps to
    old key  (kt//2)*256 + 2q + (kt%2)."""
    perm = np.empty(KH, np.int64)
    for pair in range(NP):
        base = pair * 256
        perm[pair * 256 : pair * 256 + 128] = base + np.arange(0, 256, 2)
        perm[pair * 256 + 128 : pair * 256 + 256] = base + np.arange(1, 256, 2)
    return perm


def _host_prep(boundary_map, uncertainty_map, key_w1, bn_scale, bn_bias,
               bn_mean, bn_var, key_w2, query_w, value_w):
    import ml_dtypes

    bf16 = ml_dtypes.bfloat16
    b, c, h, w = uncertainty_map.shape
    H0 = boundary_map.shape[2]
    idx = (np.arange(h) * H0) // h
    bm = boundary_map[:, 0][:, idx][:, :, idx].reshape(b, h * w).astype(np.float32)

    inv = bn_scale / np.sqrt(bn_var + 1e-5)
    beta = (bn_bias - bn_mean * inv).astype(np.float32)
    kw1f = (key_w1[:, 0] * inv).astype(np.float32)
    m_t = np.ascontiguousarray((key_w2.T @ query_w).T).astype(np.float32)  # [256, 64]
    # duplicate across partition halves for the energy duo-packing
    kw1f2 = np.concatenate([kw1f, kw1f]).reshape(1, 2 * MID).astype(bf16)
    beta2 = np.concatenate([beta, beta]).reshape(2 * MID, 1).astype(np.float32)
    m_t2 = np.concatenate([m_t, m_t], axis=1).astype(bf16)                 # [256, 128]
    vw_t = np.ascontiguousarray(value_w.T).astype(bf16)                    # [256, 256]
    perm = _kperm()

    in_maps = []
    for core in range(8):
        bi, kh = core // 2, core % 2
        u = np.ascontiguousarray(uncertainty_map[bi].reshape(c, h * w)).astype(bf16)
        uk = u[:, kh * KH : (kh + 1) * KH][:, perm]
        bmk = bm[bi, kh * KH : (kh + 1) * KH][perm]
        in_maps.append({
            "u_in": u,
            "uk_in": np.ascontiguousarray(uk),
            "bmk_in": np.ascontiguousarray(bmk).reshape(1, KH).astype(bf16),
            "mt_in": m_t2,
            "vwt_in": vw_t,
            "kw1f_in": kw1f2,
            "beta_in": beta2,
        })
    return in_maps


def kernel(boundary_map, uncertainty_map, key_w1, bn_scale, bn_bias,
           bn_mean, bn_var, key_w2, query_w, value_w, gamma):
    global LAST_RESULTS
    from concourse.bass_utils import run_bass_kernel_spmd

    nc = _get_built()
    in_maps = _host_prep(
        np.asarray(boundary_map), np.asarray(uncertainty_map), np.asarray(key_w1),
        np.asarray(bn_scale), np.asarray(bn_bias), np.asarray(bn_mean),
        np.asarray(bn_var), np.asarray(key_w2), np.asarray(query_w),
        np.asarray(value_w),
    )
    kwargs = {}
    if TRACE:
        kwargs["trace"] = True
        if TRACE_CORES is not None:
            kwargs["trace_cores"] = TRACE_CORES
    res = run_bass_kernel_spmd(nc, in_maps, core_ids=list(range(8)), **kwargs)
    LAST_RESULTS = res

    b, c, h, w = uncertainty_map.shape
    g = np.float32(np.asarray(gamma).reshape(-1)[0] / VSCALE)
    out = np.empty((b, c, h * w), np.float32)
    um = np.asarray(uncertainty_map)
    for bi in range(b):
        P = res.results[2 * bi]["outp"] + res.results[2 * bi + 1]["outp"]
        out[bi] = g * P + um[bi].reshape(c, h * w)
    return out.reshape(b, c, h, w)



# revision 11
# speedup vs baseline: 1.2463x; 1.2463x over previous
"""BoundaryAttentionModule Trainium2 kernel — moment-expansion algorithm.

Shapes (hardcoded): b=4, c=256, h=w=64 (HW=4096), boundary 128x128,
mid=64, out_ch=256. 8 cores: core = (batch bi = core//2, key-half kh = core%2).

Math: keys K[:,k] = W2 @ relu(kw1f * t_k + beta) depend on the SCALAR
boundary value t_k, so within each linear region S of the 64-breakpoint
piecewise map, E^T[k,j] = t_k * A_S[j] + B_S[j] where
  A_S[j] = sum_{i in S} tau*kw1f_i G[i,j],  B_S[j] = sum_{i in S} beta_i G[i,j]
(G = (W2^T Q_w) @ u, tau = max|t| folded into A). Then exactly
  U[k,j] = exp(B_S[j]) * sum_n ((t_k/tau)^n) * (A_S[j])^n / n!
so with moments Mo[(n,S),c] = sum_{k in S} (t_k/tau)^n v[k,c]/s_k and
w[(n,S),j] = exp(B_S[j]) A_S[j]^n/n!, the attention output collapses to
  P[c,j] = sum_{(n,S)} Mo[(n,S),c] w[(n,S),j].
Softmax denominators s[k] = sum_n (t_k/tau)^n sigma[n,S(k)] with
sigma = row-sums of w (free via accum_out). Truncation N=5 gives ~1e-5
final rel err (|tau*A| <~ 0.7). Regions padded/merged to RP=32.

Device per core: G2 = M@u, AB = CAB@G2 (duo-packed), w-chain on DVE,
vT = u_k^T@vw^T, s tiny matmuls, PW*(1/s), moment matmul, P matmul.
Host: region construction, (t/tau)^n powers, final gamma*P+u.
"""

import numpy as np

B, C, HW = 4, 256, 4096
KH = HW // 2          # 2048 keys per core
NKT = KH // 128       # 16 key tiles
MID = 64
RP = 32               # padded region count
NC = 6                # Taylor orders 0..5
COLS = RP * NC        # 192 (n-major: row n*RP+S); tiles: 128 + 64

TRACE = False
TRACE_CORES = None
LAST_RESULTS = None

_BUILT = None


def _build():
    import concourse.bass as bass
    import concourse.tile as tile
    from concourse import bacc, mybir

    f32 = mybir.dt.float32
    bf16 = mybir.dt.bfloat16
    AF = mybir.ActivationFunctionType
    AX = mybir.AxisListType
    ALU = mybir.AluOpType

    nc = bacc.Bacc(
        "TRN2",
        target_bir_lowering=False,
        debug=False,
        enable_asserts=False,
        num_devices=8,
    )

    u_in = nc.dram_tensor("u_in", [C, HW], bf16, kind="ExternalInput").ap()
    mt_in = nc.dram_tensor("mt_in", [C, 2 * MID], bf16, kind="ExternalInput").ap()
    # cols 0:128 = 4 replicas of tau*CA^T (so A lands in all four 32-row
    # partition groups), cols 128:160 = CB^T; rows duplicated for duo-packing.
    cab_in = nc.dram_tensor("cab_in", [2 * MID, 5 * RP], bf16, kind="ExternalInput").ap()
    vwt_in = nc.dram_tensor("vwt_in", [C, C], bf16, kind="ExternalInput").ap()
    pw_in = nc.dram_tensor("pw_in", [KH, COLS], bf16, kind="ExternalInput").ap()
    pwt_in = nc.dram_tensor("pwt_in", [COLS, KH], bf16, kind="ExternalInput").ap()
    p_out = nc.dram_tensor("p_out", [C, HW], bf16, kind="ExternalOutput").ap()

    NJC = 8
    JW = HW // NJC        # 512-wide j chunks
    NCC = 2
    CW = HW // NCC        # 2048-wide chain chunks

    with tile.TileContext(nc) as tc:
        with (
            tc.tile_pool(name="sb", bufs=1) as sb,
            tc.tile_pool(name="ost", bufs=2) as osp,
            tc.tile_pool(name="big", bufs=2, space="PSUM") as bigp,
            tc.tile_pool(name="ab", bufs=2, space="PSUM") as abp,
            tc.tile_pool(name="pin", bufs=1, space="PSUM") as pinp,
        ):
            # ---- input DMAs ----
            mt = sb.tile([128, 2 * MID], bf16, tag="mt", name="mt")
            nc.sync.dma_start(mt[:], mt_in[0:128, :])
            mt1 = sb.tile([128, 2 * MID], bf16, tag="mt1", name="mt1")
            nc.sync.dma_start(mt1[:], mt_in[128:256, :])
            cab = sb.tile([2 * MID, 5 * RP], bf16, tag="cab", name="cab")
            nc.sync.dma_start(cab[:], cab_in[:, :])
            u0 = sb.tile([128, HW], bf16, tag="u0", name="u0")
            u1 = sb.tile([128, HW], bf16, tag="u1", name="u1")
            for jc in range(4):
                jo = jc * 1024
                nc.sync.dma_start(u0[:, jo : jo + 1024], u_in[0:128, jo : jo + 1024])
                nc.sync.dma_start(u1[:, jo : jo + 1024], u_in[128:256, jo : jo + 1024])
            vwt0 = sb.tile([128, C], bf16, tag="vwt0", name="vwt0")
            nc.gpsimd.dma_start(vwt0[:], vwt_in[0:128, :])
            vwt1 = sb.tile([128, C], bf16, tag="vwt1", name="vwt1")
            nc.gpsimd.dma_start(vwt1[:], vwt_in[128:256, :])
            pwsb = sb.tile([128, NKT * COLS], bf16, tag="pwsb", name="pwsb")
            for kt in range(NKT):
                nc.gpsimd.dma_start(
                    pwsb[:, kt * COLS : (kt + 1) * COLS],
                    pw_in[kt * 128 : (kt + 1) * 128, :],
                )
            pwt0 = sb.tile([128, KH], bf16, tag="pwt0", name="pwt0")
            nc.gpsimd.dma_start(pwt0[:], pwt_in[0:128, :])
            pwt1 = sb.tile([64, KH], bf16, tag="pwt1", name="pwt1")
            nc.gpsimd.dma_start(pwt1[:], pwt_in[128:192, :])

            # ---- SBUF working tiles ----
            G2 = sb.tile([128, HW], bf16, tag="G2", name="G2")
            # tau*A replicated into all four 32-row partition groups so the
            # chain's two SBUF inputs always share a base partition.
            AsclR = sb.tile([128, HW], bf16, tag="AsclR", name="AsclR")
            W0 = sb.tile([128, HW], bf16, tag="W0", name="W0")   # n=0..3
            W1 = sb.tile([64, HW], bf16, tag="W1", name="W1")    # n=4,5
            sacc0 = sb.tile([128, NJC], f32, tag="sacc0", name="sacc0")
            sacc1 = sb.tile([64, NJC], f32, tag="sacc1", name="sacc1")
            nc.gpsimd.memset(sacc0[:], 0.0)
            nc.gpsimd.memset(sacc1[:], 0.0)
            sig0 = sb.tile([128, 1], f32, tag="sig0", name="sig0")
            sig1 = sb.tile([64, 1], f32, tag="sig1", name="sig1")
            sigb0 = sb.tile([128, 1], bf16, tag="sigb0", name="sigb0")
            sigb1 = sb.tile([64, 1], bf16, tag="sigb1", name="sigb1")
            rinv = sb.tile([128, NKT], f32, tag="rinv", name="rinv")
            vtb = sb.tile([128, NKT * C], bf16, tag="vtb", name="vtb")
            pws = sb.tile([128, NKT * COLS], bf16, tag="pws", name="pws")
            mo0 = sb.tile([128, C], bf16, tag="mo0", name="mo0")
            mo1 = sb.tile([64, C], bf16, tag="mo1", name="mo1")

            spin = pinp.tile([128, 512], f32, tag="spin", name="spin")
            s_ps = spin[:, 0:NKT]                  # s accumulators
            mo_ps0 = spin[:, 256 : 256 + C]        # Mo0 accumulation
            # Mo1 lives in a separate small psum region (vt pool tile pinned)
            mo_ps1t = pinp.tile([64, 256], f32, tag="mo1p", name="mo_ps1")
            mo_ps1 = mo_ps1t[:, 0:C]

            # ---- per 512 j-chunk: G2 matmul+copy, A/B matmuls, expB, AsclR ----
            def g2_chunk(jc):
                jo = jc * JW
                pg = bigp.tile([128, JW], f32, tag="big", name=f"pg{jc}")
                nc.tensor.matmul(
                    pg[:], mt[:, :], u0[:, jo : jo + JW], start=True, stop=False
                )
                nc.tensor.matmul(
                    pg[:], mt1[:, :], u1[:, jo : jo + JW], start=False, stop=True
                )
                nc.vector.tensor_copy(G2[:, jo : jo + JW], pg[:, 0:JW])

            def ab_chunk(jc):
                # alternate G2 partition halves per chunk -> duo concurrency
                jo = jc * JW
                hb = 64 * (jc % 2)
                pa = abp.tile([128, JW], f32, tag="paA", name=f"paA{jc}")
                nc.tensor.matmul(
                    pa[:], cab[hb : hb + 64, 0:128],
                    G2[hb : hb + 64, jo : jo + JW], start=True, stop=True,
                )
                pb = abp.tile([32, JW], f32, tag="paB", name=f"paB{jc}")
                nc.tensor.matmul(
                    pb[:], cab[hb : hb + 64, 128:160],
                    G2[hb : hb + 64, jo : jo + JW], start=True, stop=True,
                )
                nc.scalar.activation(
                    W0[0:32, jo : jo + JW], pb[0:32, 0:JW], AF.Exp,
                    accum_out=sacc0[0:32, jc : jc + 1],
                )
                nc.vector.tensor_copy(AsclR[:, jo : jo + JW], pa[:, 0:JW])

            def chain_step(n, cc):
                jo = cc * CW
                if n < 4:
                    dst = W0[n * 32 : (n + 1) * 32, jo : jo + CW]
                    acc = sacc0[n * 32 : (n + 1) * 32, cc : cc + 1]
                else:
                    dst = W1[(n - 4) * 32 : (n - 3) * 32, jo : jo + CW]
                    acc = sacc1[(n - 4) * 32 : (n - 3) * 32, cc : cc + 1]
                if n - 1 < 4:
                    src = W0[(n - 1) * 32 : n * 32, jo : jo + CW]
                    rep = AsclR[(n - 1) * 32 : n * 32, jo : jo + CW]
                else:
                    src = W1[(n - 5) * 32 : (n - 4) * 32, jo : jo + CW]
                    rep = AsclR[(n - 5) * 32 : (n - 4) * 32, jo : jo + CW]
                nc.vector.scalar_tensor_tensor(
                    dst, src, 1.0 / n, rep,
                    op0=ALU.mult, op1=ALU.mult, accum_out=acc,
                )

            # ---- vT matmuls (independent of chain; fills PE) ----
            def vt_tile(kt):
                pv = abp.tile([128, JW], f32, tag="paA", name=f"pv{kt}")
                ko = kt * 128
                nc.tensor.matmul(
                    pv[:, 0:C], u0[:, ko : ko + 128], vwt0[:],
                    start=True, stop=False,
                )
                nc.tensor.matmul(
                    pv[:, 0:C], u1[:, ko : ko + 128], vwt1[:],
                    start=False, stop=True,
                )
                nc.scalar.copy(vtb[:, kt * C : (kt + 1) * C], pv[:, 0:C])

            # Emission order drives the Tile scheduler's priorities.
            for jc in range(NJC):
                g2_chunk(jc)
                ab_chunk(jc)
            # chain: 2048-wide chunks so step (1, cc=0) starts at half-way
            for cc in range(NCC):
                for n in range(1, NC):
                    chain_step(n, cc)
            # vT work interleaves via scheduler (no deps on chain).
            # Keys are always u columns 0..KH-1: the host rolls u's j axis
            # per core so its key half leads, and un-rolls P afterward.
            for kt in range(NKT):
                vt_tile(kt)

            # ---- sigma -> s -> rinv -> PW scale ----
            nc.vector.reduce_sum(sig0[:], sacc0[:], axis=AX.X)
            nc.vector.reduce_sum(sig1[:], sacc1[:], axis=AX.X)
            nc.vector.tensor_copy(sigb0[:], sig0[:])
            nc.vector.tensor_copy(sigb1[:], sig1[:])
            for kt in range(NKT):
                nc.tensor.matmul(
                    s_ps[:, kt : kt + 1],
                    pwt0[:, kt * 128 : (kt + 1) * 128], sigb0[:],
                    start=True, stop=False,
                )
                nc.tensor.matmul(
                    s_ps[:, kt : kt + 1],
                    pwt1[:, kt * 128 : (kt + 1) * 128], sigb1[:],
                    start=False, stop=True,
                )
            nc.vector.reciprocal(rinv[:], s_ps[:])
            for kt in range(NKT):
                nc.vector.tensor_scalar(
                    pws[:, kt * COLS : (kt + 1) * COLS],
                    pwsb[:, kt * COLS : (kt + 1) * COLS],
                    rinv[:, kt : kt + 1], None, op0=ALU.mult,
                )

            # ---- moment matmul: Mo[(n,S), c] ----
            for kt in range(NKT):
                st, sp = kt == 0, kt == NKT - 1
                nc.tensor.matmul(
                    mo_ps0[:],
                    pws[:, kt * COLS : kt * COLS + 128],
                    vtb[:, kt * C : (kt + 1) * C],
                    start=st, stop=sp,
                )
                nc.tensor.matmul(
                    mo_ps1[:],
                    pws[:, kt * COLS + 128 : (kt + 1) * COLS],
                    vtb[:, kt * C : (kt + 1) * C],
                    start=st, stop=sp,
                )
            nc.scalar.copy(mo0[:], mo_ps0[:])
            nc.vector.tensor_copy(mo1[:], mo_ps1[:])

            # ---- P = Mo^T @ W -> DRAM ----
            for ct in range(2):
                for jc in range(NJC):
                    jo = jc * JW
                    pp = bigp.tile([128, JW], f32, tag="big", name=f"pp{ct}_{jc}")
                    for q in range(JW // 512):
                        sl = slice(q * 512, (q + 1) * 512)
                        js = jo + q * 512
                        nc.tensor.matmul(
                            pp[:, sl],
                            mo0[:, ct * 128 : (ct + 1) * 128],
                            W0[:, js : js + 512],
                            start=True, stop=False,
                        )
                        nc.tensor.matmul(
                            pp[:, sl],
                            mo1[:, ct * 128 : (ct + 1) * 128],
                            W1[:, js : js + 512],
                            start=False, stop=True,
                        )
                    ost = osp.tile([128, JW], bf16, tag="ost", name=f"ost{ct}_{jc}")
                    h = JW // 2
                    nc.scalar.copy(ost[:, 0:h], pp[:, 0:h])
                    nc.vector.tensor_copy(ost[:, h:JW], pp[:, h:JW])
                    nc.scalar.dma_start(
                        p_out[ct * 128 : (ct + 1) * 128, jo : jo + JW], ost[:, 0:JW]
                    )

    nc.compile()
    return nc


def _get_built():
    global _BUILT
    if _BUILT is None:
        _BUILT = _build()
    return _BUILT


def _regions(kw1f, beta, tmin, tmax):
    """Region edges (sorted breakpoints in range, capped at RP-1) and the
    per-region active-set midpoints."""
    bp = -beta / np.where(np.abs(kw1f) < 1e-30, 1e-30, kw1f)
    inr = np.sort(bp[(bp > tmin) & (bp < tmax)])
    while len(inr) > RP - 1:       # merge narrowest adjacent regions
        gaps = np.diff(np.concatenate([[tmin], inr, [tmax]]))
        i = int(np.argmin(gaps[:-1] + gaps[1:]))
        inr = np.delete(inr, i)
    full = np.concatenate([[tmin - 1.0], inr, [tmax + 1.0]])
    tmid = 0.5 * (full[:-1] + full[1:])
    return inr, tmid


def _host_prep(boundary_map, uncertainty_map, key_w1, bn_scale, bn_bias,
               bn_mean, bn_var, key_w2, query_w, value_w):
    import ml_dtypes

    bf = ml_dtypes.bfloat16
    b, c, h, w = uncertainty_map.shape
    H0 = boundary_map.shape[2]
    idx = (np.arange(h) * H0) // h
    bm = boundary_map[:, 0][:, idx][:, :, idx].reshape(b, h * w).astype(np.float64)

    inv = bn_scale.astype(np.float64) / np.sqrt(bn_var.astype(np.float64) + 1e-5)
    beta = bn_bias.astype(np.float64) - bn_mean.astype(np.float64) * inv
    kw1f = key_w1[:, 0].astype(np.float64) * inv
    m_t = np.ascontiguousarray((key_w2.T @ query_w).T).astype(np.float64)  # [256, 64]
    m_t2 = np.concatenate([m_t, m_t], axis=1).astype(bf)                   # [256, 128]
    vw_t = np.ascontiguousarray(value_w.T).astype(bf)                      # [256, 256]

    in_maps = []
    for core in range(8):
        bi, kh = core // 2, core % 2
        t_full = bm[bi]
        tau = np.abs(t_full).max()
        edges, tmid = _regions(kw1f, beta, t_full.min(), t_full.max())
        R = len(edges) + 1
        masks = (kw1f[None, :] * tmid[:, None] + beta[None, :]) > 0   # [R, 64]
        ca = (masks * kw1f[None, :]) * tau                            # [R, 64]
        cb = masks * beta[None, :]
        cabm = np.zeros((MID, 5 * RP), np.float64)
        for r in range(4):                    # 4 replicas of tau*CA^T
            cabm[:, r * RP : r * RP + R] = ca.T
        cabm[:, 4 * RP : 4 * RP + R] = cb.T
        cab2 = np.concatenate([cabm, cabm], axis=0).astype(bf)        # [128, 160]

        tk = t_full[kh * KH : (kh + 1) * KH]
        reg = np.searchsorted(edges, tk)                              # [2048]
        tp = np.empty((NC, KH), np.float64)
        tp[0] = 1.0
        for n in range(1, NC):
            tp[n] = tp[n - 1] * (tk / tau)
        pw = np.zeros((KH, COLS), np.float64)
        pw[np.arange(KH)[None, :].repeat(NC, 0).ravel(),
           (np.arange(NC)[:, None] * RP + reg[None, :]).ravel()] = tp.ravel()

        u = uncertainty_map[bi].reshape(c, h * w)
        u = np.ascontiguousarray(np.roll(u, -kh * KH, axis=1)).astype(bf)
        in_maps.append({
            "u_in": u,
            "mt_in": m_t2,
            "cab_in": cab2,
            "vwt_in": vw_t,
            "pw_in": pw.astype(bf),
            "pwt_in": np.ascontiguousarray(pw.T).astype(bf),
        })
    return in_maps


def kernel(boundary_map, uncertainty_map, key_w1, bn_scale, bn_bias,
           bn_mean, bn_var, key_w2, query_w, value_w, gamma):
    global LAST_RESULTS
    from concourse.bass_utils import run_bass_kernel_spmd

    nc = _get_built()
    in_maps = _host_prep(
        np.asarray(boundary_map), np.asarray(uncertainty_map), np.asarray(key_w1),
        np.asarray(bn_scale), np.asarray(bn_bias), np.asarray(bn_mean),
        np.asarray(bn_var), np.asarray(key_w2), np.asarray(query_w),
        np.asarray(value_w),
    )
    kwargs = {}
    if TRACE:
        kwargs["trace"] = True
        if TRACE_CORES is not None:
            kwargs["trace_cores"] = TRACE_CORES
    res = run_bass_kernel_spmd(nc, in_maps, core_ids=list(range(8)), **kwargs)
    LAST_RESULTS = res

    b, c, h, w = uncertainty_map.shape
    g = np.float32(np.asarray(gamma).reshape(-1)[0])
    out = np.empty((b, c, h * w), np.float32)
    um = np.asarray(uncertainty_map)
    for bi in range(b):
        P = (res.results[2 * bi]["p_out"].astype(np.float32)
             + np.roll(res.results[2 * bi + 1]["p_out"].astype(np.float32),
                       KH, axis=1))
        out[bi] = g * P + um[bi].reshape(c, h * w)
    return out.reshape(b, c, h, w)


# revision 24
# speedup vs baseline: 1.6957x; 1.3606x over previous
"""BoundaryAttentionModule Trainium2 kernel — moment-expansion algorithm.

Shapes (hardcoded): b=4, c=256, h=w=64 (HW=4096), boundary 128x128,
mid=64, out_ch=256. 8 cores: core = (batch bi = core//2, key-half kh = core%2).

Math: keys K[:,k] = W2 @ relu(kw1f * t_k + beta) depend on the SCALAR
boundary value t_k, so within each linear region S of the 64-breakpoint
piecewise map, E^T[k,j] = t_k * A_S[j] + B_S[j] where
  A_S[j] = sum_{i in S} tau*kw1f_i G[i,j],  B_S[j] = sum_{i in S} beta_i G[i,j]
(G = (W2^T Q_w) @ u, tau = max|t| folded into A). Then exactly
  U[k,j] = exp(B_S[j]) * sum_n ((t_k/tau)^n) * (A_S[j])^n / n!
so with moments Mo[(n,S),c] = sum_{k in S} (t_k/tau)^n v[k,c]/s_k and
w[(n,S),j] = exp(B_S[j]) A_S[j]^n/n!, the attention output collapses to
  P[c,j] = sum_{(n,S)} Mo[(n,S),c] w[(n,S),j].
Softmax denominators s[k] = sum_n (t_k/tau)^n sigma[n,S(k)] with
sigma = row-sums of w (free via accum_out). Truncation N=3 is exact to
well below the bf16 noise floor (final rel err ~1.4e-5; |tau*A| <~ 0.7).
Regions padded/merged to RP=32, so (n,S) = 4*32 = 128 = one partition tile.

Device per core: G2 = M@u, A/B = CAB@G2 (duo across alternating halves),
3-step w-chain on DVE, vT = u_k^T@vw^T (fills PE under the chain),
s = PWT@sigma, PW*(1/s), moment matmul, P matmul. Host: regions,
(t/tau)^n powers, j-roll per key-half, final gamma*P+u.
"""

import numpy as np

B, C, HW = 4, 256, 4096
KH = HW // 2          # 2048 keys per core
NKT = KH // 128       # 16 key tiles
MID = 64
RP = 32               # padded region count
NC = 4                # Taylor orders 0..3
COLS = RP * NC        # 128 = one partition tile

TRACE = False
TRACE_CORES = None
LAST_RESULTS = None

_BUILT = None


def _build():
    import concourse.bass as bass
    import concourse.tile as tile
    from concourse import bacc, mybir

    f32 = mybir.dt.float32
    bf16 = mybir.dt.bfloat16
    AF = mybir.ActivationFunctionType
    AX = mybir.AxisListType
    ALU = mybir.AluOpType

    nc = bacc.Bacc(
        "TRN2",
        target_bir_lowering=False,
        debug=False,
        enable_asserts=False,
        num_devices=8,
    )

    u_in = nc.dram_tensor("u_in", [C, HW], bf16, kind="ExternalInput").ap()
    mt_in = nc.dram_tensor("mt_in", [C, 2 * MID], bf16, kind="ExternalInput").ap()
    # cols 0:128 = 4 replicas of tau*CA^T (A lands in all four 32-row
    # partition groups), cols 128:160 = CB^T; rows duplicated for duo use.
    cab_in = nc.dram_tensor("cab_in", [2 * MID, 5 * RP], bf16, kind="ExternalInput").ap()
    vwt_in = nc.dram_tensor("vwt_in", [C, C], bf16, kind="ExternalInput").ap()
    pw_in = nc.dram_tensor("pw_in", [128, NKT * COLS], bf16, kind="ExternalInput").ap()
    pwt_in = nc.dram_tensor("pwt_in", [COLS, KH], bf16, kind="ExternalInput").ap()
    p_out = nc.dram_tensor("p_out", [C, HW], bf16, kind="ExternalOutput").ap()

    NJC = 8
    JW = HW // NJC        # 512-wide j chunks

    with tile.TileContext(nc) as tc:
        with (
            tc.tile_pool(name="sb", bufs=1) as sb,
            tc.tile_pool(name="ost", bufs=2) as osp,
            tc.tile_pool(name="big", bufs=2, space="PSUM") as bigp,
            tc.tile_pool(name="ab", bufs=2, space="PSUM") as abp,
            tc.tile_pool(name="pin", bufs=1, space="PSUM") as pinp,
        ):
            # ---- input DMAs ----
            mt = sb.tile([128, 2 * MID], bf16, tag="mt", name="mt")
            nc.sync.dma_start(mt[:], mt_in[0:128, :])
            mt1 = sb.tile([128, 2 * MID], bf16, tag="mt1", name="mt1")
            nc.sync.dma_start(mt1[:], mt_in[128:256, :])
            cab = sb.tile([2 * MID, 5 * RP], bf16, tag="cab", name="cab")
            nc.sync.dma_start(cab[:], cab_in[:, :])
            u0 = sb.tile([128, HW], bf16, tag="u0", name="u0")
            u1 = sb.tile([128, HW], bf16, tag="u1", name="u1")
            for jc in range(4):
                jo = jc * 1024
                nc.sync.dma_start(u0[:, jo : jo + 1024], u_in[0:128, jo : jo + 1024])
                nc.sync.dma_start(u1[:, jo : jo + 1024], u_in[128:256, jo : jo + 1024])
            vwt0 = sb.tile([128, C], bf16, tag="vwt0", name="vwt0")
            nc.gpsimd.dma_start(vwt0[:], vwt_in[0:128, :])
            vwt1 = sb.tile([128, C], bf16, tag="vwt1", name="vwt1")
            nc.gpsimd.dma_start(vwt1[:], vwt_in[128:256, :])
            pwsb = sb.tile([128, NKT * COLS], bf16, tag="pwsb", name="pwsb")
            nc.gpsimd.dma_start(pwsb[:], pw_in[:, :])
            pwt = sb.tile([COLS, KH], bf16, tag="pwt", name="pwt")
            nc.gpsimd.dma_start(pwt[:], pwt_in[:, :])

            # ---- SBUF working tiles ----
            G2 = sb.tile([128, HW], bf16, tag="G2", name="G2")
            # tau*A replicated into all four 32-row partition groups so the
            # chain's two SBUF inputs always share a base partition.
            AsclR = sb.tile([128, HW], bf16, tag="AsclR", name="AsclR")
            W0 = sb.tile([128, HW], bf16, tag="W0", name="W0")   # n=0..3
            sacc0 = sb.tile([128, NJC], f32, tag="sacc0", name="sacc0")
            nc.gpsimd.memset(sacc0[:], 0.0)
            sig0 = sb.tile([128, 1], f32, tag="sig0", name="sig0")
            sigb0 = sb.tile([128, 1], bf16, tag="sigb0", name="sigb0")
            rinv = sb.tile([128, NKT], f32, tag="rinv", name="rinv")
            vtb = sb.tile([128, NKT * C], bf16, tag="vtb", name="vtb")
            pws = sb.tile([128, NKT * COLS], bf16, tag="pws", name="pws")
            mo0 = sb.tile([128, C], bf16, tag="mo0", name="mo0")
            kwsrc = sb.tile([32, NC], bf16, tag="kwsrc", name="kwsrc")

            spin = pinp.tile([128, 512], f32, tag="spin", name="spin")
            s_ps = spin[:, 0:NKT]                  # s accumulators
            mo_ps0 = spin[:, 256 : 256 + C]        # Mo accumulation

            # ---- per 512 j-chunk: G2 matmul+copy, A/B matmuls, expB, AsclR ----
            def g2_chunk(jc):
                jo = jc * JW
                pg = bigp.tile([128, JW], f32, tag="big", name=f"pg{jc}")
                nc.tensor.matmul(
                    pg[:], mt[:, :], u0[:, jo : jo + JW], start=True, stop=False
                )
                nc.tensor.matmul(
                    pg[:], mt1[:, :], u1[:, jo : jo + JW], start=False, stop=True
                )
                nc.vector.tensor_copy(G2[:, jo : jo + JW], pg[:, 0:JW])

            def ab_chunk(jc):
                # alternate G2 partition halves per chunk -> duo concurrency
                jo = jc * JW
                hb = 64 * (jc % 2)
                pa = abp.tile([128, JW], f32, tag="paA", name=f"paA{jc}")
                nc.tensor.matmul(
                    pa[:], cab[hb : hb + 64, 0:128],
                    G2[hb : hb + 64, jo : jo + JW], start=True, stop=True,
                )
                pb = abp.tile([32, JW], f32, tag="paB", name=f"paB{jc}")
                nc.tensor.matmul(
                    pb[:], cab[hb : hb + 64, 128:160],
                    G2[hb : hb + 64, jo : jo + JW], start=True, stop=True,
                )
                nc.scalar.activation(
                    W0[0:32, jo : jo + JW], pb[0:32, 0:JW], AF.Exp,
                    accum_out=sacc0[0:32, jc : jc + 1],
                )
                # alternate the 4-replica copy between ACT and DVE per chunk
                if jc % 2 == 0:
                    nc.scalar.copy(AsclR[:, jo : jo + JW], pa[:, 0:JW])
                else:
                    nc.vector.tensor_copy(AsclR[:, jo : jo + JW], pa[:, 0:JW])

            def chain_step(n):
                dst = W0[n * 32 : (n + 1) * 32, :]
                src = W0[(n - 1) * 32 : n * 32, :]
                rep = AsclR[(n - 1) * 32 : n * 32, :]
                nc.vector.scalar_tensor_tensor(
                    dst, src, 1.0 / n, rep,
                    op0=ALU.mult, op1=ALU.mult,
                    accum_out=sacc0[n * 32 : (n + 1) * 32, 0:1],
                )

            def keep_warm(n):
                # Tiny matmul data-dependent on chain step n (via a 1-col DVE
                # copy to a base-0 tile): spaces PE activity through the chain
                # so HAM stays at K=8/8.
                nc.vector.tensor_copy(
                    kwsrc[:, n : n + 1], W0[n * 32 : n * 32 + 32, 0:1]
                )
                pz = abp.tile([32, JW], f32, tag="paB", name=f"kw{n}")
                nc.tensor.matmul(
                    pz[0:1, 0:64], kwsrc[:, n : n + 1], u0[0:32, 0:64],
                    start=True, stop=True,
                )

            # ---- vT matmuls (independent of chain; fills PE) ----
            def vt_tile(kt):
                pv = abp.tile([128, JW], f32, tag="paA", name=f"pv{kt}")
                ko = kt * 128
                nc.tensor.matmul(
                    pv[:, 0:C], u0[:, ko : ko + 128], vwt0[:],
                    start=True, stop=False,
                )
                nc.tensor.matmul(
                    pv[:, 0:C], u1[:, ko : ko + 128], vwt1[:],
                    start=False, stop=True,
                )
                nc.scalar.copy(vtb[:, kt * C : (kt + 1) * C], pv[:, 0:C])

            # Emission order drives the Tile scheduler's priorities.
            for jc in range(NJC):
                g2_chunk(jc)
                ab_chunk(jc)
            # vT first in the PE FIFO: fills the PE while the chain runs.
            # Keys are always u columns 0..KH-1: the host rolls u's j axis
            # per core so its key half leads, and un-rolls P afterward.
            for kt in range(NKT):
                vt_tile(kt)
            # chain on DVE; keep-warm matmuls bridge the PE through its tail
            for n in range(1, NC):
                chain_step(n)
                keep_warm(n)

            # ---- sigma -> s -> rinv ----
            nc.vector.reduce_sum(sig0[:], sacc0[:], axis=AX.X)
            nc.vector.tensor_copy(sigb0[:], sig0[:])
            for kt in range(NKT):
                nc.tensor.matmul(
                    s_ps[:, kt : kt + 1],
                    pwt[:, kt * 128 : (kt + 1) * 128], sigb0[:],
                    start=True, stop=True,
                )
            nc.vector.reciprocal(rinv[:], s_ps[:])

            # ---- pws scaling interleaved with the moment matmul ----
            for kt in range(NKT):
                nc.vector.tensor_scalar(
                    pws[:, kt * COLS : (kt + 1) * COLS],
                    pwsb[:, kt * COLS : (kt + 1) * COLS],
                    rinv[:, kt : kt + 1], None, op0=ALU.mult,
                )
                nc.tensor.matmul(
                    mo_ps0[:],
                    pws[:, kt * COLS : (kt + 1) * COLS],
                    vtb[:, kt * C : (kt + 1) * C],
                    start=(kt == 0), stop=(kt == NKT - 1),
                )
            nc.scalar.copy(mo0[:], mo_ps0[:])

            # ---- P = Mo^T @ W -> DRAM (1024-wide output groups) ----
            for ct in range(2):
                for jg in range(4):
                    jo = jg * 1024
                    ost = osp.tile([128, 1024], bf16, tag="ost", name=f"ost{ct}_{jg}")
                    for q in range(2):
                        sl = slice(q * 512, (q + 1) * 512)
                        js = jo + q * 512
                        pp = bigp.tile([128, JW], f32, tag="big",
                                       name=f"pp{ct}_{jg}_{q}")
                        nc.tensor.matmul(
                            pp[:],
                            mo0[:, ct * 128 : (ct + 1) * 128],
                            W0[:, js : js + 512],
                            start=True, stop=True,
                        )
                        if q == 0:
                            nc.scalar.copy(ost[:, sl], pp[:])
                        else:
                            nc.vector.tensor_copy(ost[:, sl], pp[:])
                    nc.sync.dma_start(
                        p_out[ct * 128 : (ct + 1) * 128, jo : jo + 1024],
                        ost[:, 0:1024],
                    )

    nc.compile()
    return nc


def _get_built():
    global _BUILT
    if _BUILT is None:
        _BUILT = _build()
    return _BUILT


def _regions(kw1f, beta, tmin, tmax):
    """Region edges (sorted breakpoints in range, capped at RP-1) and the
    per-region active-set midpoints."""
    bp = -beta / np.where(np.abs(kw1f) < 1e-30, 1e-30, kw1f)
    inr = np.sort(bp[(bp > tmin) & (bp < tmax)])
    while len(inr) > RP - 1:       # merge narrowest adjacent regions
        gaps = np.diff(np.concatenate([[tmin], inr, [tmax]]))
        i = int(np.argmin(gaps[:-1] + gaps[1:]))
        inr = np.delete(inr, i)
    full = np.concatenate([[tmin - 1.0], inr, [tmax + 1.0]])
    tmid = 0.5 * (full[:-1] + full[1:])
    return inr, tmid


def _host_prep(boundary_map, uncertainty_map, key_w1, bn_scale, bn_bias,
               bn_mean, bn_var, key_w2, query_w, value_w):
    import ml_dtypes

    bf = ml_dtypes.bfloat16
    b, c, h, w = uncertainty_map.shape
    H0 = boundary_map.shape[2]
    idx = (np.arange(h) * H0) // h
    bm = boundary_map[:, 0][:, idx][:, :, idx].reshape(b, h * w).astype(np.float64)

    inv = bn_scale.astype(np.float64) / np.sqrt(bn_var.astype(np.float64) + 1e-5)
    beta = bn_bias.astype(np.float64) - bn_mean.astype(np.float64) * inv
    kw1f = key_w1[:, 0].astype(np.float64) * inv
    m_t = np.ascontiguousarray((key_w2.T @ query_w).T).astype(np.float64)  # [256, 64]
    m_t2 = np.concatenate([m_t, m_t], axis=1).astype(bf)                   # [256, 128]
    vw_t = np.ascontiguousarray(value_w.T).astype(bf)                      # [256, 256]

    in_maps = []
    for core in range(8):
        bi, kh = core // 2, core % 2
        t_full = bm[bi]
        tau = np.abs(t_full).max()
        edges, tmid = _regions(kw1f, beta, t_full.min(), t_full.max())
        R = len(edges) + 1
        masks = (kw1f[None, :] * tmid[:, None] + beta[None, :]) > 0   # [R, 64]
        ca = (masks * kw1f[None, :]) * tau                            # [R, 64]
        cb = masks * beta[None, :]
        cabm = np.zeros((MID, 5 * RP), np.float64)
        for r in range(4):                    # 4 replicas of tau*CA^T
            cabm[:, r * RP : r * RP + R] = ca.T
        cabm[:, 4 * RP : 4 * RP + R] = cb.T
        cab2 = np.concatenate([cabm, cabm], axis=0).astype(bf)        # [128, 160]

        tk = t_full[kh * KH : (kh + 1) * KH]
        reg = np.searchsorted(edges, tk)                              # [2048]
        tp = np.empty((NC, KH), np.float64)
        tp[0] = 1.0
        for n in range(1, NC):
            tp[n] = tp[n - 1] * (tk / tau)
        pw = np.zeros((KH, COLS), np.float64)
        pw[np.arange(KH)[None, :].repeat(NC, 0).ravel(),
           (np.arange(NC)[:, None] * RP + reg[None, :]).ravel()] = tp.ravel()
        # device layout: [128, NKT*COLS] (k-tile t at cols t*COLS)
        pw_dev = pw.reshape(NKT, 128, COLS).transpose(1, 0, 2).reshape(128, NKT * COLS)

        u = uncertainty_map[bi].reshape(c, h * w)
        u = np.ascontiguousarray(np.roll(u, -kh * KH, axis=1)).astype(bf)
        in_maps.append({
            "u_in": u,
            "mt_in": m_t2,
            "cab_in": cab2,
            "vwt_in": vw_t,
            "pw_in": np.ascontiguousarray(pw_dev).astype(bf),
            "pwt_in": np.ascontiguousarray(pw.T).astype(bf),
        })
    return in_maps


def kernel(boundary_map, uncertainty_map, key_w1, bn_scale, bn_bias,
           bn_mean, bn_var, key_w2, query_w, value_w, gamma):
    global LAST_RESULTS
    from concourse.bass_utils import run_bass_kernel_spmd

    nc = _get_built()
    in_maps = _host_prep(
        np.asarray(boundary_map), np.asarray(uncertainty_map), np.asarray(key_w1),
        np.asarray(bn_scale), np.asarray(bn_bias), np.asarray(bn_mean),
        np.asarray(bn_var), np.asarray(key_w2), np.asarray(query_w),
        np.asarray(value_w),
    )
    kwargs = {}
    if TRACE:
        kwargs["trace"] = True
        if TRACE_CORES is not None:
            kwargs["trace_cores"] = TRACE_CORES
    res = run_bass_kernel_spmd(nc, in_maps, core_ids=list(range(8)), **kwargs)
    LAST_RESULTS = res

    b, c, h, w = uncertainty_map.shape
    g = np.float32(np.asarray(gamma).reshape(-1)[0])
    out = np.empty((b, c, h * w), np.float32)
    um = np.asarray(uncertainty_map)
    for bi in range(b):
        P = (res.results[2 * bi]["p_out"].astype(np.float32)
             + np.roll(res.results[2 * bi + 1]["p_out"].astype(np.float32),
                       KH, axis=1))
        out[bi] = g * P + um[bi].reshape(c, h * w)
    return out.reshape(b, c, h, w)
